# revision 19
# baseline (speedup 1.0000x reference)
"""Multi-head attention Bass/Tile kernel for 8 TRN2 NeuronCores.

Problem: nn_MultiHeadAttention (B=4, T1=T2=2048, d_model=256, d_key=32, H=8,
per-head value dim = d_model).  Reference math (no score scaling, no mask):

    k = key   @ WK^T + bk           [B, T1, 256]   (head h -> cols 32h..32h+32)
    q = query @ WQ^T + bq           [B, T2, 256]
    v = value @ WV^T + bv           [B, T1, 2048]  (head h -> cols 256h..256h+256)
    scores_h = k_h q_h^T            [T1, T2]
    attn = softmax over T1 (keys)
    emb_h = attn^T v_h              [T2, 256]
    out = emb' @ WO^T + bo          emb' channel c = d*8 + h (d outer, h inner)

Host-side weight folding (graph-compiler style, done once in numpy):
  - M_h = WV_h^T WO_h^T  [256, 256] per head -> M [256, 2048]; then
    out[q,:] = sum_h attn_h^T u_h + cvec  with  u_h = value @ M_h and
    cvec = bo + sum_h WO_h bv_h   (softmax rows sum to 1, so the v-bias
    contribution is constant and folds into cvec).
  - Inputs are transposed to channel-major and cast to bf16 on the host
    (layout/dtype assignment), so the device does no transposes or casts.

Sharding: core c handles (batch b = c//2, query half qs = c%2) -> each core
computes the full output slice out[b, qs*1024:(qs+1)*1024, :].  No collectives.

Per-core algorithm (all matmuls bf16 with fp32 PSUM accumulation):
  - kT[c,s] = wkT^T keyT (+bk), qT[c,q] likewise              (PE+ACT)
  - u[s, c'] = valT^T M, stored head-interleaved with a ones column per
    head: u cols h*257..h*257+255 = data, col h*257+256 = 1.0  (PE+DVE)
  - per head h: scores_h[s,q] = kT_h^T qT_h -> PSUM, E = exp  (PE+ACT)
  - per (h, qt): Pbar[q, 0:257] = sum_s E_h[s, q-tile] * [u_h | 1]
    -> col 256 is the softmax denominator for free           (PE)
    acc[q,:] (+)= Pbar[:,0:256] * (1/Pbar[:,256])  (+cvec at h=0) (DVE)
  - scores(h+1) interleaved with attn(h) so ACT exp hides behind PE.

kernel(**inputs) takes the FULL unsharded inputs and returns the full output.
"""

import numpy as np
import ml_dtypes
from contextlib import ExitStack

import concourse.bass as bass
import concourse.bacc as bacc
import concourse.mybir as mybir
import concourse.tile as tile
from concourse.bass_utils import run_bass_kernel_spmd

P = 128
B, T1, T2, DM, DK, H = 4, 2048, 2048, 256, 32, 8
QSH = T2 // 2  # queries per core
N_CORES = 8

F32 = mybir.dt.float32
BF16 = mybir.dt.bfloat16
AF = mybir.ActivationFunctionType

ST = T1 // P        # 16 key/seq tiles
QT = QSH // P       # 8 query tiles per core
UW = DM + 1         # 257: per-head u block width (data + ones column)


def _build_bass():
    nc = bacc.Bacc("TRN2", target_bir_lowering=False, debug=False)

    keyt = nc.dram_tensor("keyt", [DM, T1], BF16, kind="ExternalInput").ap()
    qryt = nc.dram_tensor("qryt", [DM, QSH], BF16, kind="ExternalInput").ap()
    valt = nc.dram_tensor("valt", [DM, T1], BF16, kind="ExternalInput").ap()
    wkt = nc.dram_tensor("wkt", [DM, DM], BF16, kind="ExternalInput").ap()
    wqt = nc.dram_tensor("wqt", [DM, DM], BF16, kind="ExternalInput").ap()
    wkb = nc.dram_tensor("wkb", [DM], F32, kind="ExternalInput").ap()
    wqb = nc.dram_tensor("wqb", [DM], F32, kind="ExternalInput").ap()
    mw = nc.dram_tensor("mw", [DM, H * DM], BF16, kind="ExternalInput").ap()
    cvec = nc.dram_tensor("cvec", [DM], F32, kind="ExternalInput").ap()
    out = nc.dram_tensor("out_y", [QSH, DM], F32, kind="ExternalOutput").ap()

    with tile.TileContext(nc, pool_alloc_mode="queue") as tc:
        with ExitStack() as ctx:
            _body(ctx, tc, keyt, qryt, valt, wkt, wqt, wkb, wqb, mw, cvec, out)
    nc.compile()
    return nc


def _body(ctx, tc, keyt, qryt, valt, wkt, wqt, wkb, wqb, mw, cvec, out):
    nc = tc.nc
    mult, add = mybir.AluOpType.mult, mybir.AluOpType.add

    consts = ctx.enter_context(tc.tile_pool(name="consts", bufs=1))
    main = ctx.enter_context(tc.tile_pool(name="main", bufs=1))
    # One PSUM pool, 8 banks via 2 tags:
    #   tag S: [128,1024] x2 (4 banks)  scores tiles only
    #   tag A: [128, 512] x4 (4 banks)  k/q proj + u proj + attn accumulators
    # Slot-reuse distance (PE work between allocs of the same slot) must
    # exceed the drain chain (~0.9us DVE/ACT copy, ~1.3us exp): S reuses
    # 2 allocs apart (>=2.5us of PE), A 4 apart (>=1.7us).  Any PE stall
    # also resets the p-state clock ramp, so zero-stall matters double.
    pP = ctx.enter_context(tc.tile_pool(name="pP", bufs=1, space="PSUM"))

    # biases; wk_b[p, t] = wkb[t*128+p] so kT tile ct gets bias wk_b[:, ct]
    wk_b = consts.tile([P, 2], F32)
    nc.gpsimd.dma_start(out=wk_b, in_=wkb.rearrange("(t p) -> p t", p=P))
    wq_b = consts.tile([P, 2], F32)
    nc.gpsimd.dma_start(out=wq_b, in_=wqb.rearrange("(t p) -> p t", p=P))
    # constant output vector broadcast along partitions (step-0 partition AP)
    cvec_bc = consts.tile([P, DM], F32)
    nc.gpsimd.dma_start(
        out=cvec_bc,
        in_=bass.AP(tensor=cvec.tensor, offset=cvec.offset, ap=[[0, P], [1, DM]]),
    )

    kT = main.tile([P, 2, T1], BF16)       # [c, s]
    qT = main.tile([P, 2, QSH], BF16)      # [c, q]
    u = main.tile([P, ST, H * UW], BF16)   # [s, h*257+d], col h*257+256 = 1.0
    acc = main.tile([P, QT, DM], F32)      # output accumulator [q, cout]

    with ExitStack() as s0:
        stg = s0.enter_context(tc.tile_pool(name="stg", bufs=1))
        sE = s0.enter_context(tc.tile_pool(name="sE", bufs=2))
        srec = s0.enter_context(tc.tile_pool(name="srec", bufs=4))

        # ---------------- loads (already transposed/cast on host) ----------
        # valt quarters on the SP queue, m quarters on the ACT queue (their
        # fixed DGE overheads overlap; u projection consumes both first),
        # then the k/q path on SP.
        valt_s = stg.tile([P, 2, T1], BF16)
        valt_r = valt.rearrange("(t p) s -> p t s", p=P)
        m_s = stg.tile([P, 2, H * DM], BF16)
        m_r = mw.rearrange("(t p) c -> p t c", p=P)
        for qtr in range(4):
            sl = slice(qtr * 512, (qtr + 1) * 512)
            nc.sync.dma_start(out=valt_s[:, :, sl], in_=valt_r[:, :, sl])
            nc.scalar.dma_start(out=m_s[:, :, sl], in_=m_r[:, :, sl])
        wkt_s = stg.tile([P, 2, DM], BF16)
        nc.sync.dma_start(out=wkt_s, in_=wkt.rearrange("(t p) c -> p t c", p=P))
        keyt_s = stg.tile([P, 2, T1], BF16)
        keyt_r = keyt.rearrange("(t p) s -> p t s", p=P)
        for half in range(2):
            sl = slice(half * (T1 // 2), (half + 1) * (T1 // 2))
            nc.sync.dma_start(out=keyt_s[:, :, sl], in_=keyt_r[:, :, sl])
        wqt_s = stg.tile([P, 2, DM], BF16)
        nc.sync.dma_start(out=wqt_s, in_=wqt.rearrange("(t p) c -> p t c", p=P))
        qryt_s = stg.tile([P, 2, QSH], BF16)
        nc.sync.dma_start(out=qryt_s, in_=qryt.rearrange("(t p) q -> p t q", p=P))

        # ones columns of u (one per head)
        for h in range(H):
            nc.vector.memset(u[:, :, h * UW + DM:h * UW + DM + 1], 1.0)

        def proj_psum(name):
            return pP.tile([P, 512], F32, tag="A", name=name, bufs=4)

        # ---------------- k/q projections ----------------------------------
        # kT[c, s] = sum_d wkT[d, c] keyT[d, s]  (+bias, ACT/DVE alternating)
        def proj_unit(i, dst, w_s, x_s, b_col, ct, sc):
            pp = proj_psum(f"pp{i}")
            for dt in range(2):
                nc.tensor.matmul(pp, w_s[:, dt, ct * P:(ct + 1) * P],
                                 x_s[:, dt, sc * 512:(sc + 1) * 512],
                                 start=(dt == 0), stop=(dt == 1))
            out_sl = dst[:, ct, sc * 512:(sc + 1) * 512]
            if i % 2 == 0:
                nc.scalar.activation(out=out_sl, in_=pp,
                                     func=AF.Identity, bias=b_col)
            else:
                # op1=bypass ignores in1, but it must not be PSUM; use an
                # SBUF operand this unit already depends on
                nc.vector.scalar_tensor_tensor(
                    out=out_sl, in0=pp, scalar=b_col,
                    in1=x_s[:, 0, sc * 512:(sc + 1) * 512],
                    op0=add, op1=mybir.AluOpType.bypass)

        # ---------------- u projection -------------------------------------
        # u[s, c] = sum_d valT[d, s] M[d, c]; psum chunk cc covers heads
        # 2cc, 2cc+1; the drain writes it head-interleaved (stride 257).
        def u_unit(cc, st, drain):
            pu = proj_psum(f"pu{cc}_{st}")
            for dt in range(2):
                nc.tensor.matmul(pu, valt_s[:, dt, st * P:(st + 1) * P],
                                 m_s[:, dt, cc * 512:(cc + 1) * 512],
                                 start=(dt == 0), stop=(dt == 1))
            dst = u[:, st, cc * 2 * UW:(cc + 1) * 2 * UW]
            dst = dst.rearrange("p (h c) -> p h c", c=UW)[:, :, 0:DM]
            src = pu.rearrange("p (h c) -> p h c", c=DM)
            if drain == "act":
                nc.scalar.copy(out=dst, in_=src)
            else:
                nc.vector.tensor_copy(out=dst, in_=src)

        # ---------------- attention ----------------------------------------
        Es = {}

        def scores_unit(h, st):
            """scores_h[s-tile, :] -> PSUM -> E via ACT exp."""
            if st == 0:
                Es[h] = sE.tile([P, ST, QSH], BF16, tag="E", name=f"E{h}")
            base, ctile = 32 * (h % 4), h // 4
            ps = pP.tile([P, QSH], F32, tag="S", name=f"sc{h}_{st}", bufs=2)
            for qc in range(2):
                nc.tensor.matmul(
                    ps[:, qc * 512:(qc + 1) * 512],
                    kT[base:base + 32, ctile, st * P:(st + 1) * P],
                    qT[base:base + 32, ctile, qc * 512:(qc + 1) * 512],
                    start=True, stop=True, tile_position=(base, 0))
            nc.scalar.activation(out=Es[h][:, st, :], in_=ps, func=AF.Exp)

        def attn_unit(h, qt):
            """Pbar[q,0:257] = sum_s E_h^T [u_h | 1]; scale+accumulate."""
            pb = pP.tile([P, 512], F32, tag="A", name=f"pb{h}_{qt}", bufs=4)
            for st in range(ST):
                nc.tensor.matmul(pb[:, 0:UW],
                                 Es[h][:, st, qt * P:(qt + 1) * P],
                                 u[:, st, h * UW:(h + 1) * UW],
                                 start=(st == 0), stop=(st == ST - 1))
            rec = srec.tile([P, 1], F32, tag="r", name=f"rec{h}_{qt}")
            nc.vector.reciprocal(out=rec, in_=pb[:, DM:DM + 1])
            nc.vector.scalar_tensor_tensor(
                out=acc[:, qt, :], in0=pb[:, 0:DM], scalar=rec,
                in1=(cvec_bc if h == 0 else acc[:, qt, :]),
                op0=mult, op1=add)
            if h == H - 1:
                # Pool/SWDGE queue: shorter fixed chain than HWDGE for the
                # final store, which sits on the kernel's tail
                nc.gpsimd.dma_start(
                    out=out.rearrange("(n p) d -> p n d", p=P)[:, qt, :],
                    in_=acc[:, qt, :])

        # Emission order: u chunk 0 (while k/q inputs still loading), k/q
        # projections, then u chunks 1-3 with scores(0) spread 1-per-3 so
        # head 0's exp (16.6us of ACT) hides behind the u matmuls.
        for st in range(ST):
            u_unit(0, st, drain=("act", "dve")[st % 2])
        i = 0
        for ct in range(2):
            for sc in range(T1 // 512):
                proj_unit(i, kT, wkt_s, keyt_s, wk_b[:, ct:ct + 1], ct, sc)
                i += 1
        for ct in range(2):
            for sc in range(QSH // 512):
                proj_unit(i, qT, wqt_s, qryt_s, wq_b[:, ct:ct + 1], ct, sc)
                i += 1
        nu, sc0 = 0, 0
        for cc in range(1, 4):
            for st in range(ST):
                # ACT does exp(0) here; keep 2/3 of the u drains on DVE
                u_unit(cc, st, drain=("dve", "act", "dve")[nu % 3])
                nu += 1
                if nu % 3 == 0 and sc0 < ST:
                    scores_unit(0, sc0)
                    sc0 += 1

        # steady state: scores(h+1) interleaved with attn(h)
        for h in range(H):
            for qt in range(QT):
                if h + 1 < H:
                    scores_unit(h + 1, 2 * qt)
                    scores_unit(h + 1, 2 * qt + 1)
                attn_unit(h, qt)


_NC_CACHE = None


def _get_nc():
    global _NC_CACHE
    if _NC_CACHE is None:
        _NC_CACHE = _build_bass()
    return _NC_CACHE


def _fold_weights(inputs):
    """Host-side constant folding: M = blockdiag-ish fold of WV and WO,
    cvec = all output-side biases (softmax rows sum to 1)."""
    f32 = lambda x: np.asarray(x, dtype=np.float32)
    WV_w, WV_b = f32(inputs["WV_w"]), f32(inputs["WV_b"])
    WO_w, WO_b = f32(inputs["WO_w"]), f32(inputs["WO_b"])
    M = np.empty((DM, H * DM), dtype=np.float32)
    cv = WO_b.copy()
    for h in range(H):
        WVh = WV_w[h * DM:(h + 1) * DM, :]        # [256 vdim, 256 din]
        Wth = WO_w[:, h::H]                       # [256 out, 256 vdim]
        M[:, h * DM:(h + 1) * DM] = WVh.T @ Wth.T
        cv += Wth @ WV_b[h * DM:(h + 1) * DM]
    return M, cv


def _make_in_maps(inputs):
    bf = lambda x: np.ascontiguousarray(np.asarray(x, dtype=np.float32)).astype(
        ml_dtypes.bfloat16)
    f32c = lambda x: np.ascontiguousarray(np.asarray(x, dtype=np.float32))
    M, cv = _fold_weights(inputs)
    shared = {
        "wkt": bf(np.asarray(inputs["WK_w"], dtype=np.float32).T),
        "wqt": bf(np.asarray(inputs["WQ_w"], dtype=np.float32).T),
        "wkb": f32c(inputs["WK_b"]),
        "wqb": f32c(inputs["WQ_b"]),
        "mw": bf(M),
        "cvec": f32c(cv),
    }
    key_in = np.asarray(inputs["key_input"], dtype=np.float32)
    qry_in = np.asarray(inputs["query_input"], dtype=np.float32)
    val_in = np.asarray(inputs["value_input"], dtype=np.float32)
    in_maps = []
    for c in range(N_CORES):
        b, qs = c // 2, c % 2
        in_maps.append(dict(
            shared,
            keyt=bf(key_in[b].T),
            qryt=bf(qry_in[b, qs * QSH:(qs + 1) * QSH].T),
            valt=bf(val_in[b].T),
        ))
    return in_maps


def _assemble(results):
    out = np.empty((B, T2, DM), dtype=np.float32)
    for c in range(N_CORES):
        b, qs = c // 2, c % 2
        out[b, qs * QSH:(qs + 1) * QSH] = results[c]["out_y"]
    return out


def run_spmd(inputs, **kwargs):
    """Run the kernel on all 8 cores; kwargs forwarded (e.g. trace=True)."""
    nc = _get_nc()
    res = run_bass_kernel_spmd(nc, _make_in_maps(inputs),
                               core_ids=list(range(N_CORES)), **kwargs)
    return res


def kernel(**inputs):
    res = run_spmd(inputs)
    return _assemble(res.results)


# revision 20
# speedup vs baseline: 1.0053x; 1.0053x over previous
"""Multi-head attention Bass/Tile kernel for 8 TRN2 NeuronCores.

Problem: nn_MultiHeadAttention (B=4, T1=T2=2048, d_model=256, d_key=32, H=8,
per-head value dim = d_model).  Reference math (no score scaling, no mask):

    k = key   @ WK^T + bk           [B, T1, 256]   (head h -> cols 32h..32h+32)
    q = query @ WQ^T + bq           [B, T2, 256]
    v = value @ WV^T + bv           [B, T1, 2048]  (head h -> cols 256h..256h+256)
    scores_h = k_h q_h^T            [T1, T2]
    attn = softmax over T1 (keys)
    emb_h = attn^T v_h              [T2, 256]
    out = emb' @ WO^T + bo          emb' channel c = d*8 + h (d outer, h inner)

Host-side weight folding (graph-compiler style, done once in numpy):
  - M_h = WV_h^T WO_h^T  [256, 256] per head -> M [256, 2048]; then
    out[q,:] = sum_h attn_h^T u_h + cvec  with  u_h = value @ M_h and
    cvec = bo + sum_h WO_h bv_h   (softmax rows sum to 1, so the v-bias
    contribution is constant and folds into cvec).
  - Inputs are transposed to channel-major and cast to bf16 on the host
    (layout/dtype assignment), so the device does no transposes or casts.

Sharding: core c handles (batch b = c//2, query half qs = c%2) -> each core
computes the full output slice out[b, qs*1024:(qs+1)*1024, :].  No collectives.

Per-core algorithm (all matmuls bf16 with fp32 PSUM accumulation):
  - kT[c,s] = wkT^T keyT (+bk), qT[c,q] likewise              (PE+ACT)
  - u[s, c'] = valT^T M, stored head-interleaved with a ones column per
    head: u cols h*257..h*257+255 = data, col h*257+256 = 1.0  (PE+DVE)
  - per head h: scores_h[s,q] = kT_h^T qT_h -> PSUM, E = exp  (PE+ACT)
  - per (h, qt): Pbar[q, 0:257] = sum_s E_h[s, q-tile] * [u_h | 1]
    -> col 256 is the softmax denominator for free           (PE)
    acc[q,:] (+)= Pbar[:,0:256] * (1/Pbar[:,256])  (+cvec at h=0) (DVE)
  - scores(h+1) interleaved with attn(h) so ACT exp hides behind PE.

kernel(**inputs) takes the FULL unsharded inputs and returns the full output.
"""

import numpy as np
import ml_dtypes
from contextlib import ExitStack

import concourse.bass as bass
import concourse.bacc as bacc
import concourse.mybir as mybir
import concourse.tile as tile
from concourse.bass_utils import run_bass_kernel_spmd

P = 128
B, T1, T2, DM, DK, H = 4, 2048, 2048, 256, 32, 8
QSH = T2 // 2  # queries per core
N_CORES = 8

F32 = mybir.dt.float32
BF16 = mybir.dt.bfloat16
AF = mybir.ActivationFunctionType

ST = T1 // P        # 16 key/seq tiles
QT = QSH // P       # 8 query tiles per core
UW = DM + 1         # 257: per-head u block width (data + ones column)


def _build_bass():
    nc = bacc.Bacc("TRN2", target_bir_lowering=False, debug=False)

    keyt = nc.dram_tensor("keyt", [DM, T1], BF16, kind="ExternalInput").ap()
    qryt = nc.dram_tensor("qryt", [DM, QSH], BF16, kind="ExternalInput").ap()
    valt = nc.dram_tensor("valt", [DM, T1], BF16, kind="ExternalInput").ap()
    wkt = nc.dram_tensor("wkt", [DM, DM], BF16, kind="ExternalInput").ap()
    wqt = nc.dram_tensor("wqt", [DM, DM], BF16, kind="ExternalInput").ap()
    wkb = nc.dram_tensor("wkb", [DM], F32, kind="ExternalInput").ap()
    wqb = nc.dram_tensor("wqb", [DM], F32, kind="ExternalInput").ap()
    mw = nc.dram_tensor("mw", [DM, H * DM], BF16, kind="ExternalInput").ap()
    cvec = nc.dram_tensor("cvec", [DM], F32, kind="ExternalInput").ap()
    out = nc.dram_tensor("out_y", [QSH, DM], F32, kind="ExternalOutput").ap()

    with tile.TileContext(nc, pool_alloc_mode="queue") as tc:
        with ExitStack() as ctx:
            _body(ctx, tc, keyt, qryt, valt, wkt, wqt, wkb, wqb, mw, cvec, out)
    nc.compile()
    return nc


def _body(ctx, tc, keyt, qryt, valt, wkt, wqt, wkb, wqb, mw, cvec, out):
    nc = tc.nc
    mult, add = mybir.AluOpType.mult, mybir.AluOpType.add

    consts = ctx.enter_context(tc.tile_pool(name="consts", bufs=1))
    main = ctx.enter_context(tc.tile_pool(name="main", bufs=1))
    # One PSUM pool, 8 banks via 2 tags:
    #   tag S: [128,1024] x2 (4 banks)  scores tiles only
    #   tag A: [128, 512] x4 (4 banks)  k/q proj + u proj + attn accumulators
    # Slot-reuse distance (PE work between allocs of the same slot) must
    # exceed the drain chain (~0.9us DVE/ACT copy, ~1.3us exp): S reuses
    # 2 allocs apart (>=2.5us of PE), A 4 apart (>=1.7us).  Any PE stall
    # also resets the p-state clock ramp, so zero-stall matters double.
    pP = ctx.enter_context(tc.tile_pool(name="pP", bufs=1, space="PSUM"))

    # biases; wk_b[p, t] = wkb[t*128+p] so kT tile ct gets bias wk_b[:, ct]
    wk_b = consts.tile([P, 2], F32)
    nc.gpsimd.dma_start(out=wk_b, in_=wkb.rearrange("(t p) -> p t", p=P))
    wq_b = consts.tile([P, 2], F32)
    nc.gpsimd.dma_start(out=wq_b, in_=wqb.rearrange("(t p) -> p t", p=P))
    # constant output vector broadcast along partitions (step-0 partition AP)
    cvec_bc = consts.tile([P, DM], F32)
    nc.gpsimd.dma_start(
        out=cvec_bc,
        in_=bass.AP(tensor=cvec.tensor, offset=cvec.offset, ap=[[0, P], [1, DM]]),
    )

    kT = main.tile([P, 2, T1], BF16)       # [c, s]
    qT = main.tile([P, 2, QSH], BF16)      # [c, q]
    u = main.tile([P, ST, H * UW], BF16)   # [s, h*257+d], col h*257+256 = 1.0
    acc = main.tile([P, QT, DM], F32)      # output accumulator [q, cout]

    with ExitStack() as s0:
        stg = s0.enter_context(tc.tile_pool(name="stg", bufs=1))
        sE = s0.enter_context(tc.tile_pool(name="sE", bufs=2))
        srec = s0.enter_context(tc.tile_pool(name="srec", bufs=4))

        # ---------------- loads (already transposed/cast on host) ----------
        # valt quarters on the SP queue, m quarters on the ACT queue (their
        # fixed DGE overheads overlap; u projection consumes both first),
        # then the k/q path on SP.
        valt_s = stg.tile([P, 2, T1], BF16)
        valt_r = valt.rearrange("(t p) s -> p t s", p=P)
        m_s = stg.tile([P, 2, H * DM], BF16)
        m_r = mw.rearrange("(t p) c -> p t c", p=P)
        # small first valt piece so the first u matmul fires ~0.4us sooner
        nc.sync.dma_start(out=valt_s[:, :, 0:P], in_=valt_r[:, :, 0:P])
        for qtr in range(4):
            sl = slice(max(qtr * 512, P), (qtr + 1) * 512)
            nc.sync.dma_start(out=valt_s[:, :, sl], in_=valt_r[:, :, sl])
            nc.scalar.dma_start(out=m_s[:, :, slice(qtr * 512, (qtr + 1) * 512)],
                                in_=m_r[:, :, slice(qtr * 512, (qtr + 1) * 512)])
        wkt_s = stg.tile([P, 2, DM], BF16)
        nc.sync.dma_start(out=wkt_s, in_=wkt.rearrange("(t p) c -> p t c", p=P))
        keyt_s = stg.tile([P, 2, T1], BF16)
        keyt_r = keyt.rearrange("(t p) s -> p t s", p=P)
        for half in range(2):
            sl = slice(half * (T1 // 2), (half + 1) * (T1 // 2))
            nc.sync.dma_start(out=keyt_s[:, :, sl], in_=keyt_r[:, :, sl])
        wqt_s = stg.tile([P, 2, DM], BF16)
        nc.sync.dma_start(out=wqt_s, in_=wqt.rearrange("(t p) c -> p t c", p=P))
        qryt_s = stg.tile([P, 2, QSH], BF16)
        nc.sync.dma_start(out=qryt_s, in_=qryt.rearrange("(t p) q -> p t q", p=P))

        # ones columns of u (one per head)
        for h in range(H):
            nc.vector.memset(u[:, :, h * UW + DM:h * UW + DM + 1], 1.0)

        def proj_psum(name):
            return pP.tile([P, 512], F32, tag="A", name=name, bufs=4)

        # ---------------- k/q projections ----------------------------------
        # kT[c, s] = sum_d wkT[d, c] keyT[d, s]  (+bias, ACT/DVE alternating)
        def proj_unit(i, dst, w_s, x_s, b_col, ct, sc):
            pp = proj_psum(f"pp{i}")
            for dt in range(2):
                nc.tensor.matmul(pp, w_s[:, dt, ct * P:(ct + 1) * P],
                                 x_s[:, dt, sc * 512:(sc + 1) * 512],
                                 start=(dt == 0), stop=(dt == 1))
            out_sl = dst[:, ct, sc * 512:(sc + 1) * 512]
            if i % 2 == 0:
                nc.scalar.activation(out=out_sl, in_=pp,
                                     func=AF.Identity, bias=b_col)
            else:
                # op1=bypass ignores in1, but it must not be PSUM; use an
                # SBUF operand this unit already depends on
                nc.vector.scalar_tensor_tensor(
                    out=out_sl, in0=pp, scalar=b_col,
                    in1=x_s[:, 0, sc * 512:(sc + 1) * 512],
                    op0=add, op1=mybir.AluOpType.bypass)

        # ---------------- u projection -------------------------------------
        # u[s, c] = sum_d valT[d, s] M[d, c]; psum chunk cc covers heads
        # 2cc, 2cc+1; the drain writes it head-interleaved (stride 257).
        def u_unit(cc, st, drain):
            pu = proj_psum(f"pu{cc}_{st}")
            for dt in range(2):
                nc.tensor.matmul(pu, valt_s[:, dt, st * P:(st + 1) * P],
                                 m_s[:, dt, cc * 512:(cc + 1) * 512],
                                 start=(dt == 0), stop=(dt == 1))
            dst = u[:, st, cc * 2 * UW:(cc + 1) * 2 * UW]
            dst = dst.rearrange("p (h c) -> p h c", c=UW)[:, :, 0:DM]
            src = pu.rearrange("p (h c) -> p h c", c=DM)
            if drain == "act":
                nc.scalar.copy(out=dst, in_=src)
            else:
                nc.vector.tensor_copy(out=dst, in_=src)

        # ---------------- attention ----------------------------------------
        Es = {}

        def scores_unit(h, st):
            """scores_h[s-tile, :] -> PSUM -> E via ACT exp."""
            if st == 0:
                Es[h] = sE.tile([P, ST, QSH], BF16, tag="E", name=f"E{h}")
            base, ctile = 32 * (h % 4), h // 4
            ps = pP.tile([P, QSH], F32, tag="S", name=f"sc{h}_{st}", bufs=2)
            for qc in range(2):
                nc.tensor.matmul(
                    ps[:, qc * 512:(qc + 1) * 512],
                    kT[base:base + 32, ctile, st * P:(st + 1) * P],
                    qT[base:base + 32, ctile, qc * 512:(qc + 1) * 512],
                    start=True, stop=True, tile_position=(base, 0))
            nc.scalar.activation(out=Es[h][:, st, :], in_=ps, func=AF.Exp)

        def attn_unit(h, qt):
            """Pbar[q,0:257] = sum_s E_h^T [u_h | 1]; scale+accumulate."""
            pb = pP.tile([P, 512], F32, tag="A", name=f"pb{h}_{qt}", bufs=4)
            for st in range(ST):
                nc.tensor.matmul(pb[:, 0:UW],
                                 Es[h][:, st, qt * P:(qt + 1) * P],
                                 u[:, st, h * UW:(h + 1) * UW],
                                 start=(st == 0), stop=(st == ST - 1))
            rec = srec.tile([P, 1], F32, tag="r", name=f"rec{h}_{qt}")
            nc.vector.reciprocal(out=rec, in_=pb[:, DM:DM + 1])
            nc.vector.scalar_tensor_tensor(
                out=acc[:, qt, :], in0=pb[:, 0:DM], scalar=rec,
                in1=(cvec_bc if h == 0 else acc[:, qt, :]),
                op0=mult, op1=add)
            if h == H - 1:
                # Pool/SWDGE queue: shorter fixed chain than HWDGE for the
                # final store, which sits on the kernel's tail
                nc.gpsimd.dma_start(
                    out=out.rearrange("(n p) d -> p n d", p=P)[:, qt, :],
                    in_=acc[:, qt, :])

        # Emission order: u chunk 0 (while k/q inputs still loading), k/q
        # projections, then u chunks 1-3 with scores(0) spread 1-per-3 so
        # head 0's exp (16.6us of ACT) hides behind the u matmuls.
        for st in range(ST):
            u_unit(0, st, drain=("act", "dve")[st % 2])
        i = 0
        for ct in range(2):
            for sc in range(T1 // 512):
                proj_unit(i, kT, wkt_s, keyt_s, wk_b[:, ct:ct + 1], ct, sc)
                i += 1
        for ct in range(2):
            for sc in range(QSH // 512):
                proj_unit(i, qT, wqt_s, qryt_s, wq_b[:, ct:ct + 1], ct, sc)
                i += 1
        nu, sc0 = 0, 0
        for cc in range(1, 4):
            for st in range(ST):
                # ACT does exp(0) here; keep 2/3 of the u drains on DVE
                u_unit(cc, st, drain=("dve", "act", "dve")[nu % 3])
                nu += 1
                if nu % 3 == 0 and sc0 < ST:
                    scores_unit(0, sc0)
                    sc0 += 1

        # steady state: scores(h+1) interleaved with attn(h)
        for h in range(H):
            for qt in range(QT):
                if h + 1 < H:
                    scores_unit(h + 1, 2 * qt)
                    scores_unit(h + 1, 2 * qt + 1)
                attn_unit(h, qt)


_NC_CACHE = None


def _get_nc():
    global _NC_CACHE
    if _NC_CACHE is None:
        _NC_CACHE = _build_bass()
    return _NC_CACHE


def _fold_weights(inputs):
    """Host-side constant folding: M = blockdiag-ish fold of WV and WO,
    cvec = all output-side biases (softmax rows sum to 1)."""
    f32 = lambda x: np.asarray(x, dtype=np.float32)
    WV_w, WV_b = f32(inputs["WV_w"]), f32(inputs["WV_b"])
    WO_w, WO_b = f32(inputs["WO_w"]), f32(inputs["WO_b"])
    M = np.empty((DM, H * DM), dtype=np.float32)
    cv = WO_b.copy()
    for h in range(H):
        WVh = WV_w[h * DM:(h + 1) * DM, :]        # [256 vdim, 256 din]
        Wth = WO_w[:, h::H]                       # [256 out, 256 vdim]
        M[:, h * DM:(h + 1) * DM] = WVh.T @ Wth.T
        cv += Wth @ WV_b[h * DM:(h + 1) * DM]
    return M, cv


def _make_in_maps(inputs):
    bf = lambda x: np.ascontiguousarray(np.asarray(x, dtype=np.float32)).astype(
        ml_dtypes.bfloat16)
    f32c = lambda x: np.ascontiguousarray(np.asarray(x, dtype=np.float32))
    M, cv = _fold_weights(inputs)
    shared = {
        "wkt": bf(np.asarray(inputs["WK_w"], dtype=np.float32).T),
        "wqt": bf(np.asarray(inputs["WQ_w"], dtype=np.float32).T),
        "wkb": f32c(inputs["WK_b"]),
        "wqb": f32c(inputs["WQ_b"]),
        "mw": bf(M),
        "cvec": f32c(cv),
    }
    key_in = np.asarray(inputs["key_input"], dtype=np.float32)
    qry_in = np.asarray(inputs["query_input"], dtype=np.float32)
    val_in = np.asarray(inputs["value_input"], dtype=np.float32)
    in_maps = []
    for c in range(N_CORES):
        b, qs = c // 2, c % 2
        in_maps.append(dict(
            shared,
            keyt=bf(key_in[b].T),
            qryt=bf(qry_in[b, qs * QSH:(qs + 1) * QSH].T),
            valt=bf(val_in[b].T),
        ))
    return in_maps


def _assemble(results):
    out = np.empty((B, T2, DM), dtype=np.float32)
    for c in range(N_CORES):
        b, qs = c // 2, c % 2
        out[b, qs * QSH:(qs + 1) * QSH] = results[c]["out_y"]
    return out


def run_spmd(inputs, **kwargs):
    """Run the kernel on all 8 cores; kwargs forwarded (e.g. trace=True)."""
    nc = _get_nc()
    res = run_bass_kernel_spmd(nc, _make_in_maps(inputs),
                               core_ids=list(range(N_CORES)), **kwargs)
    return res


def kernel(**inputs):
    res = run_spmd(inputs)
    return _assemble(res.results)


# revision 22
# speedup vs baseline: 1.0080x; 1.0027x over previous
"""Multi-head attention Bass/Tile kernel for 8 TRN2 NeuronCores.

Problem: nn_MultiHeadAttention (B=4, T1=T2=2048, d_model=256, d_key=32, H=8,
per-head value dim = d_model).  Reference math (no score scaling, no mask):

    k = key   @ WK^T + bk           [B, T1, 256]   (head h -> cols 32h..32h+32)
    q = query @ WQ^T + bq           [B, T2, 256]
    v = value @ WV^T + bv           [B, T1, 2048]  (head h -> cols 256h..256h+256)
    scores_h = k_h q_h^T            [T1, T2]
    attn = softmax over T1 (keys)
    emb_h = attn^T v_h              [T2, 256]
    out = emb' @ WO^T + bo          emb' channel c = d*8 + h (d outer, h inner)

Host-side weight folding (graph-compiler style, done once in numpy):
  - M_h = WV_h^T WO_h^T  [256, 256] per head -> M [256, 2048]; then
    out[q,:] = sum_h attn_h^T u_h + cvec  with  u_h = value @ M_h and
    cvec = bo + sum_h WO_h bv_h   (softmax rows sum to 1, so the v-bias
    contribution is constant and folds into cvec).
  - Inputs are transposed to channel-major and cast to bf16 on the host
    (layout/dtype assignment), so the device does no transposes or casts.

Sharding: core c handles (batch b = c//2, query half qs = c%2) -> each core
computes the full output slice out[b, qs*1024:(qs+1)*1024, :].  No collectives.

Per-core algorithm (all matmuls bf16 with fp32 PSUM accumulation):
  - kT[c,s] = wkT^T keyT (+bk), qT[c,q] likewise              (PE+ACT)
  - u[s, c'] = valT^T M, stored head-interleaved with a ones column per
    head: u cols h*257..h*257+255 = data, col h*257+256 = 1.0  (PE+DVE)
  - per head h: scores_h[s,q] = kT_h^T qT_h -> PSUM, E = exp  (PE+ACT)
  - per (h, qt): Pbar[q, 0:257] = sum_s E_h[s, q-tile] * [u_h | 1]
    -> col 256 is the softmax denominator for free           (PE)
    acc[q,:] (+)= Pbar[:,0:256] * (1/Pbar[:,256])  (+cvec at h=0) (DVE)
  - scores(h+1) interleaved with attn(h) so ACT exp hides behind PE.

kernel(**inputs) takes the FULL unsharded inputs and returns the full output.
"""

import numpy as np
import ml_dtypes
from contextlib import ExitStack

import concourse.bass as bass
import concourse.bacc as bacc
import concourse.mybir as mybir
import concourse.tile as tile
from concourse.bass_utils import run_bass_kernel_spmd

P = 128
B, T1, T2, DM, DK, H = 4, 2048, 2048, 256, 32, 8
QSH = T2 // 2  # queries per core
N_CORES = 8

F32 = mybir.dt.float32
BF16 = mybir.dt.bfloat16
AF = mybir.ActivationFunctionType

ST = T1 // P        # 16 key/seq tiles
QT = QSH // P       # 8 query tiles per core
UW = DM + 1         # 257: per-head u block width (data + ones column)


def _build_bass():
    nc = bacc.Bacc("TRN2", target_bir_lowering=False, debug=False)

    keyt = nc.dram_tensor("keyt", [DM, T1], BF16, kind="ExternalInput").ap()
    qryt = nc.dram_tensor("qryt", [DM, QSH], BF16, kind="ExternalInput").ap()
    valt = nc.dram_tensor("valt", [DM, T1], BF16, kind="ExternalInput").ap()
    wkt = nc.dram_tensor("wkt", [DM, DM], BF16, kind="ExternalInput").ap()
    wqt = nc.dram_tensor("wqt", [DM, DM], BF16, kind="ExternalInput").ap()
    wkb = nc.dram_tensor("wkb", [DM], F32, kind="ExternalInput").ap()
    wqb = nc.dram_tensor("wqb", [DM], F32, kind="ExternalInput").ap()
    mw = nc.dram_tensor("mw", [DM, H * DM], BF16, kind="ExternalInput").ap()
    cvec = nc.dram_tensor("cvec", [DM], F32, kind="ExternalInput").ap()
    out = nc.dram_tensor("out_y", [QSH, DM], F32, kind="ExternalOutput").ap()

    with tile.TileContext(nc, pool_alloc_mode="queue") as tc:
        with ExitStack() as ctx:
            _body(ctx, tc, keyt, qryt, valt, wkt, wqt, wkb, wqb, mw, cvec, out)
    nc.compile()
    return nc


def _body(ctx, tc, keyt, qryt, valt, wkt, wqt, wkb, wqb, mw, cvec, out):
    nc = tc.nc
    mult, add = mybir.AluOpType.mult, mybir.AluOpType.add

    consts = ctx.enter_context(tc.tile_pool(name="consts", bufs=1))
    main = ctx.enter_context(tc.tile_pool(name="main", bufs=1))
    # One PSUM pool, 8 banks via 2 tags:
    #   tag S: [128,1024] x2 (4 banks)  scores tiles only
    #   tag A: [128, 512] x4 (4 banks)  k/q proj + u proj + attn accumulators
    # Slot-reuse distance (PE work between allocs of the same slot) must
    # exceed the drain chain (~0.9us DVE/ACT copy, ~1.3us exp): S reuses
    # 2 allocs apart (>=2.5us of PE), A 4 apart (>=1.7us).  Any PE stall
    # also resets the p-state clock ramp, so zero-stall matters double.
    pP = ctx.enter_context(tc.tile_pool(name="pP", bufs=1, space="PSUM"))

    # biases; wk_b[p, t] = wkb[t*128+p] so kT tile ct gets bias wk_b[:, ct]
    wk_b = consts.tile([P, 2], F32)
    nc.gpsimd.dma_start(out=wk_b, in_=wkb.rearrange("(t p) -> p t", p=P))
    wq_b = consts.tile([P, 2], F32)
    nc.gpsimd.dma_start(out=wq_b, in_=wqb.rearrange("(t p) -> p t", p=P))
    # constant output vector broadcast along partitions (step-0 partition AP)
    cvec_bc = consts.tile([P, DM], F32)
    nc.gpsimd.dma_start(
        out=cvec_bc,
        in_=bass.AP(tensor=cvec.tensor, offset=cvec.offset, ap=[[0, P], [1, DM]]),
    )

    kT = main.tile([P, 2, T1], BF16)       # [c, s]
    qT = main.tile([P, 2, QSH], BF16)      # [c, q]
    u = main.tile([P, ST, H * UW], BF16)   # [s, h*257+d], col h*257+256 = 1.0
    acc = main.tile([P, QT, DM], F32)      # output accumulator [q, cout]

    with ExitStack() as s0:
        stg = s0.enter_context(tc.tile_pool(name="stg", bufs=1))
        sE = s0.enter_context(tc.tile_pool(name="sE", bufs=2))
        srec = s0.enter_context(tc.tile_pool(name="srec", bufs=4))

        # ---------------- loads (already transposed/cast on host) ----------
        # valt quarters on the SP queue, m quarters on the ACT queue (their
        # fixed DGE overheads overlap; u projection consumes both first),
        # then the k/q path on SP.
        valt_s = stg.tile([P, 2, T1], BF16)
        valt_r = valt.rearrange("(t p) s -> p t s", p=P)
        m_s = stg.tile([P, 2, H * DM], BF16)
        m_r = mw.rearrange("(t p) c -> p t c", p=P)
        # small first valt piece so the first u matmul fires ~0.4us sooner
        nc.sync.dma_start(out=valt_s[:, :, 0:P], in_=valt_r[:, :, 0:P])
        for qtr in range(4):
            sl = slice(max(qtr * 512, P), (qtr + 1) * 512)
            nc.sync.dma_start(out=valt_s[:, :, sl], in_=valt_r[:, :, sl])
            nc.scalar.dma_start(out=m_s[:, :, slice(qtr * 512, (qtr + 1) * 512)],
                                in_=m_r[:, :, slice(qtr * 512, (qtr + 1) * 512)])
        wkt_s = stg.tile([P, 2, DM], BF16)
        nc.sync.dma_start(out=wkt_s, in_=wkt.rearrange("(t p) c -> p t c", p=P))
        keyt_s = stg.tile([P, 2, T1], BF16)
        keyt_r = keyt.rearrange("(t p) s -> p t s", p=P)
        for half in range(2):
            sl = slice(half * (T1 // 2), (half + 1) * (T1 // 2))
            nc.sync.dma_start(out=keyt_s[:, :, sl], in_=keyt_r[:, :, sl])
        wqt_s = stg.tile([P, 2, DM], BF16)
        nc.sync.dma_start(out=wqt_s, in_=wqt.rearrange("(t p) c -> p t c", p=P))
        qryt_s = stg.tile([P, 2, QSH], BF16)
        nc.sync.dma_start(out=qryt_s, in_=qryt.rearrange("(t p) q -> p t q", p=P))

        # ones columns of u (one per head)
        for h in range(H):
            nc.vector.memset(u[:, :, h * UW + DM:h * UW + DM + 1], 1.0)

        def proj_psum(name):
            return pP.tile([P, 512], F32, tag="A", name=name, bufs=4)

        # ---------------- k/q projections ----------------------------------
        # kT[c, s] = sum_d wkT[d, c] keyT[d, s]  (+bias, ACT/DVE alternating)
        def proj_unit(i, dst, w_s, x_s, b_col, ct, sc):
            pp = proj_psum(f"pp{i}")
            for dt in range(2):
                nc.tensor.matmul(pp, w_s[:, dt, ct * P:(ct + 1) * P],
                                 x_s[:, dt, sc * 512:(sc + 1) * 512],
                                 start=(dt == 0), stop=(dt == 1))
            out_sl = dst[:, ct, sc * 512:(sc + 1) * 512]
            if i % 2 == 0:
                nc.scalar.activation(out=out_sl, in_=pp,
                                     func=AF.Identity, bias=b_col)
            else:
                # op1=bypass ignores in1, but it must not be PSUM; use an
                # SBUF operand this unit already depends on
                nc.vector.scalar_tensor_tensor(
                    out=out_sl, in0=pp, scalar=b_col,
                    in1=x_s[:, 0, sc * 512:(sc + 1) * 512],
                    op0=add, op1=mybir.AluOpType.bypass)

        # ---------------- u projection -------------------------------------
        # u[s, c] = sum_d valT[d, s] M[d, c]; psum chunk cc covers heads
        # 2cc, 2cc+1; the drain writes it head-interleaved (stride 257).
        def u_unit(cc, st, drain):
            pu = proj_psum(f"pu{cc}_{st}")
            for dt in range(2):
                nc.tensor.matmul(pu, valt_s[:, dt, st * P:(st + 1) * P],
                                 m_s[:, dt, cc * 512:(cc + 1) * 512],
                                 start=(dt == 0), stop=(dt == 1))
            dst = u[:, st, cc * 2 * UW:(cc + 1) * 2 * UW]
            dst = dst.rearrange("p (h c) -> p h c", c=UW)[:, :, 0:DM]
            src = pu.rearrange("p (h c) -> p h c", c=DM)
            if drain == "act":
                nc.scalar.copy(out=dst, in_=src)
            else:
                nc.vector.tensor_copy(out=dst, in_=src)

        # ---------------- attention ----------------------------------------
        Es = {}

        def scores_unit(h, st):
            """scores_h[s-tile, :] -> PSUM -> E via ACT exp."""
            if st == 0:
                Es[h] = sE.tile([P, ST, QSH], BF16, tag="E", name=f"E{h}")
            base, ctile = 32 * (h % 4), h // 4
            ps = pP.tile([P, QSH], F32, tag="S", name=f"sc{h}_{st}", bufs=2)
            for qc in range(2):
                nc.tensor.matmul(
                    ps[:, qc * 512:(qc + 1) * 512],
                    kT[base:base + 32, ctile, st * P:(st + 1) * P],
                    qT[base:base + 32, ctile, qc * 512:(qc + 1) * 512],
                    start=True, stop=True, tile_position=(base, 0))
            nc.scalar.activation(out=Es[h][:, st, :], in_=ps, func=AF.Exp)

        def attn_unit(h, qt):
            """Pbar[q,0:257] = sum_s E_h^T [u_h | 1]; scale+accumulate."""
            pb = pP.tile([P, 512], F32, tag="A", name=f"pb{h}_{qt}", bufs=4)
            for st in range(ST):
                nc.tensor.matmul(pb[:, 0:UW],
                                 Es[h][:, st, qt * P:(qt + 1) * P],
                                 u[:, st, h * UW:(h + 1) * UW],
                                 start=(st == 0), stop=(st == ST - 1))
            rec = srec.tile([P, 1], F32, tag="r", name=f"rec{h}_{qt}")
            nc.vector.reciprocal(out=rec, in_=pb[:, DM:DM + 1])
            nc.vector.scalar_tensor_tensor(
                out=acc[:, qt, :], in0=pb[:, 0:DM], scalar=rec,
                in1=(cvec_bc if h == 0 else acc[:, qt, :]),
                op0=mult, op1=add)
            if h == H - 1:
                nc.sync.dma_start(
                    out=out.rearrange("(n p) d -> p n d", p=P)[:, qt, :],
                    in_=acc[:, qt, :])

        # Emission order: u units in (cc, st-quarter) wavefront order so
        # each unit's valt/m quarters have arrived by the time PE gets
        # there; k/q projections once keyt/qryt land; the rest of u with
        # scores(0) spread 1-per-3 so head 0's exp (16.6us of ACT) hides
        # behind the u matmuls.
        ublocks = []
        for w in range(4):
            for cc in range(4):
                for sq in range(4):
                    if max(cc, sq) == w:
                        ublocks.append((cc, sq))
        uq = [(cc, 4 * sq + j) for cc, sq in ublocks for j in range(4)]
        nu = 0

        def emit_u(n, pat):
            nonlocal nu
            for _ in range(n):
                cc, st = uq[nu]
                u_unit(cc, st, drain=pat[nu % len(pat)])
                nu += 1

        emit_u(16, ("act", "dve"))
        i = 0
        for ct in range(2):
            for sc in range(T1 // 512):
                proj_unit(i, kT, wkt_s, keyt_s, wk_b[:, ct:ct + 1], ct, sc)
                i += 1
        for ct in range(2):
            for sc in range(QSH // 512):
                proj_unit(i, qT, wqt_s, qryt_s, wq_b[:, ct:ct + 1], ct, sc)
                i += 1
        for sc0 in range(ST):
            # ACT does exp(0) here; keep 2/3 of the u drains on DVE
            emit_u(3, ("dve", "act", "dve"))
            scores_unit(0, sc0)

        # steady state: scores(h+1) interleaved with attn(h)
        for h in range(H):
            for qt in range(QT):
                if h + 1 < H:
                    scores_unit(h + 1, 2 * qt)
                    scores_unit(h + 1, 2 * qt + 1)
                attn_unit(h, qt)


_NC_CACHE = None


def _get_nc():
    global _NC_CACHE
    if _NC_CACHE is None:
        _NC_CACHE = _build_bass()
    return _NC_CACHE


def _fold_weights(inputs):
    """Host-side constant folding: M = blockdiag-ish fold of WV and WO,
    cvec = all output-side biases (softmax rows sum to 1)."""
    f32 = lambda x: np.asarray(x, dtype=np.float32)
    WV_w, WV_b = f32(inputs["WV_w"]), f32(inputs["WV_b"])
    WO_w, WO_b = f32(inputs["WO_w"]), f32(inputs["WO_b"])
    M = np.empty((DM, H * DM), dtype=np.float32)
    cv = WO_b.copy()
    for h in range(H):
        WVh = WV_w[h * DM:(h + 1) * DM, :]        # [256 vdim, 256 din]
        Wth = WO_w[:, h::H]                       # [256 out, 256 vdim]
        M[:, h * DM:(h + 1) * DM] = WVh.T @ Wth.T
        cv += Wth @ WV_b[h * DM:(h + 1) * DM]
    return M, cv


def _make_in_maps(inputs):
    bf = lambda x: np.ascontiguousarray(np.asarray(x, dtype=np.float32)).astype(
        ml_dtypes.bfloat16)
    f32c = lambda x: np.ascontiguousarray(np.asarray(x, dtype=np.float32))
    M, cv = _fold_weights(inputs)
    shared = {
        "wkt": bf(np.asarray(inputs["WK_w"], dtype=np.float32).T),
        "wqt": bf(np.asarray(inputs["WQ_w"], dtype=np.float32).T),
        "wkb": f32c(inputs["WK_b"]),
        "wqb": f32c(inputs["WQ_b"]),
        "mw": bf(M),
        "cvec": f32c(cv),
    }
    key_in = np.asarray(inputs["key_input"], dtype=np.float32)
    qry_in = np.asarray(inputs["query_input"], dtype=np.float32)
    val_in = np.asarray(inputs["value_input"], dtype=np.float32)
    in_maps = []
    for c in range(N_CORES):
        b, qs = c // 2, c % 2
        in_maps.append(dict(
            shared,
            keyt=bf(key_in[b].T),
            qryt=bf(qry_in[b, qs * QSH:(qs + 1) * QSH].T),
            valt=bf(val_in[b].T),
        ))
    return in_maps


def _assemble(results):
    out = np.empty((B, T2, DM), dtype=np.float32)
    for c in range(N_CORES):
        b, qs = c // 2, c % 2
        out[b, qs * QSH:(qs + 1) * QSH] = results[c]["out_y"]
    return out


def run_spmd(inputs, **kwargs):
    """Run the kernel on all 8 cores; kwargs forwarded (e.g. trace=True)."""
    nc = _get_nc()
    res = run_bass_kernel_spmd(nc, _make_in_maps(inputs),
                               core_ids=list(range(N_CORES)), **kwargs)
    return res


def kernel(**inputs):
    res = run_spmd(inputs)
    return _assemble(res.results)
